# revision 8
# baseline (speedup 1.0000x reference)
"""CustomLSTM (highway) Trainium2 kernel.

B=32, T=2048, I=H=256. Data-parallel over batch: 8 cores x 4 rows.
Per core: fused input projection + serial recurrence, gates on partitions.

Layouts (per core, per chunk of CH=128 timesteps):
  pichunk  SBUF [128, 12*CH*4]  bf16: col = g*(CH*4) + t*4 + b
           gate-tile order g: [m0,m1,i0,i1,f0,f1,o0,o1,hw0,hw1,x60,x61]
  ybuf     SBUF [128, (CH+1)*8] bf16: h history, col = t*8 + kh*4 + b (slot0=carry)
  cbuf     SBUF [128, (CH+1)*8] f32 : c history, same layout
  youtbuf  SBUF [128, CH*8]     f32 : exact y, col = t*8 + kh*4 + b
Recurrence matmul: out[gate_tile 128, (b)] += WsT_tile[k=128, m=128].T @ h[k=128, b=4]
pi pre-added into PSUM via identity matmul (start=True), Ws MMs accumulate.
Host extracts y[b, :len[b]], h_f = y[b, len-1], c_f from c history.
"""
import os
import sys

for _p in ("/opt/trn_rl_repo", "/root/.axon_site/_ro/trn_rl_repo"):
    if os.path.isdir(_p) and _p not in sys.path:
        sys.path.insert(0, _p)

import numpy as np
import ml_dtypes

import concourse.bass as bass
import concourse.mybir as mybir
from concourse import bacc
from concourse.tile import TileContext
from concourse.bass_utils import run_bass_kernel_spmd

BF16 = ml_dtypes.bfloat16
N_CORES = 8
B, T, I, H = 32, 2048, 256, 256
BL = B // N_CORES          # rows per core = 4
CH = 128                   # timesteps per chunk
MODE = os.environ.get("LSTM_MODE", "bf16")  # "bf16" | "fp32"

LAST_RESULTS = None        # test.py introspection


def _build(nck: int):
    fp32 = MODE == "fp32"
    wdt = mybir.dt.float32 if fp32 else mybir.dt.bfloat16
    nc = bacc.Bacc()
    ACTF = mybir.ActivationFunctionType

    xt_d = nc.declare_dram_parameter("xt", [nck, 2, 128, CH * BL], wdt, isOutput=False)
    wst_d = nc.declare_dram_parameter("wst", [128, 2560], wdt, isOutput=False)
    wit_d = nc.declare_dram_parameter("wit", [128, 3072], wdt, isOutput=False)
    bias_d = nc.declare_dram_parameter("bias", [128, 12], mybir.dt.float32, isOutput=False)
    iden_d = nc.declare_dram_parameter("iden", [128, 128], wdt, isOutput=False)
    yout_d = nc.declare_dram_parameter("yout", [nck, 128, CH * 8], mybir.dt.float32, isOutput=True)
    cout_d = nc.declare_dram_parameter("cout", [nck, 128, CH * 8], mybir.dt.float32, isOutput=True)

    f32 = mybir.dt.float32
    GW = CH * BL  # 512: per-gate-tile chunk width

    with TileContext(nc) as tc:
        with (
            tc.tile_pool(name="consts", bufs=1) as consts,
            tc.tile_pool(name="pich", bufs=2) as pich_pool,
            tc.tile_pool(name="xt", bufs=2) as xt_pool,
            tc.tile_pool(name="state", bufs=2) as state_pool,
            tc.tile_pool(name="ew", bufs=4) as ew,
            tc.tile_pool(name="g1", bufs=2, space="PSUM") as g1_pool,
            tc.tile_pool(name="g2", bufs=2, space="PSUM") as g2_pool,
            tc.tile_pool(name="g3", bufs=2, space="PSUM") as g3_pool,
            tc.tile_pool(name="pp", bufs=2, space="PSUM") as pp_pool,
        ):
            wst = consts.tile([128, 2560], wdt)
            wit = consts.tile([128, 3072], wdt)
            bias = consts.tile([128, 12], f32)
            iden = consts.tile([128, 128], wdt)
            nc.sync.dma_start(out=wst[:], in_=wst_d[:])
            nc.sync.dma_start(out=wit[:], in_=wit_d[:])
            nc.sync.dma_start(out=bias[:], in_=bias_d[:])
            nc.sync.dma_start(out=iden[:], in_=iden_d[:])

            def load_x(ck):
                xt = xt_pool.tile([128, 2 * GW], wdt, tag="xt")
                nc.sync.dma_start(out=xt[:, 0:GW], in_=xt_d[ck, 0])
                nc.sync.dma_start(out=xt[:, GW:2 * GW], in_=xt_d[ck, 1])
                return xt

            def project(xt):
                """input projection for one chunk -> pichunk tile"""
                pich = pich_pool.tile([128, 12 * GW], wdt, tag="pich")
                for g in range(12):
                    pg = pp_pool.tile([128, GW], f32, tag="pp")
                    for ih in range(2):
                        nc.tensor.matmul(
                            pg[:],
                            wit[:, (ih * 12 + g) * 128:(ih * 12 + g + 1) * 128],
                            xt[:, ih * GW:(ih + 1) * GW],
                            start=(ih == 0), stop=(ih == 1),
                        )
                    nc.vector.tensor_scalar_add(
                        out=pich[:, g * GW:(g + 1) * GW], in0=pg[:],
                        scalar1=bias[:, g:g + 1],
                    )
                return pich

            xt_cur = load_x(0)
            pich_cur = project(xt_cur)

            ybuf_prev = None
            cbuf_prev = None

            for ck in range(nck):
                ybuf = state_pool.tile([128, (CH + 1) * 8], wdt, tag="ybuf")
                cbuf = state_pool.tile([128, (CH + 1) * 8], f32, tag="cbuf")
                youtb = state_pool.tile([128, CH * 8], f32, tag="youtb")
                if ck == 0:
                    nc.vector.memset(ybuf[:, 0:8], 0.0)
                    nc.vector.memset(cbuf[:, 0:8], 0.0)
                else:
                    nc.vector.tensor_copy(out=ybuf[:, 0:8], in_=ybuf_prev[:, CH * 8:(CH + 1) * 8])
                    nc.vector.tensor_copy(out=cbuf[:, 0:8], in_=cbuf_prev[:, CH * 8:(CH + 1) * 8])

                # issue next chunk's x DMA early (completes during recurrence)
                if ck + 1 < nck:
                    xt_next = load_x(ck + 1)

                pich_r = pich_cur.rearrange("p (g t b) -> p g t b", g=12, t=CH, b=BL)

                for t in range(CH):
                    g1 = g1_pool.tile([128, 16], f32, tag="g1")   # i,f
                    g2 = g2_pool.tile([128, 8], f32, tag="g2")    # m
                    g3 = g3_pool.tile([128, 16], f32, tag="g3")   # o,hw
                    # pi pre-adds via identity matmul: all first, so the PE
                    # prefetches them while stalled on h(t-1)
                    nc.tensor.matmul(g1[:], iden[:], pich_r[:, 0:4, t, :], start=True, stop=False)
                    nc.tensor.matmul(g2[:], iden[:], pich_r[:, 4:6, t, :], start=True, stop=False)
                    nc.tensor.matmul(g3[:], iden[:], pich_r[:, 6:10, t, :], start=True, stop=False)
                    # Ws matmuls in gate order i0,i1,f0,f1 | m0,m1 | o0,o1,hw0,hw1
                    for g in range(10):
                        if g < 4:
                            dst = g1[:, g * 4:(g + 1) * 4]
                        elif g < 6:
                            dst = g2[:, (g - 4) * 4:(g - 3) * 4]
                        else:
                            dst = g3[:, (g - 6) * 4:(g - 5) * 4]
                        for kh in range(2):
                            nc.tensor.matmul(
                                dst,
                                wst[:, (kh * 10 + g) * 128:(kh * 10 + g + 1) * 128],
                                ybuf[:, t * 8 + kh * 4: t * 8 + kh * 4 + 4],
                                start=False,
                                stop=(kh == 1 and g in (3, 5, 9)),
                            )

                    sm = ew.tile([128, 8], f32, tag="sm")
                    sif = ew.tile([128, 16], f32, tag="sif")
                    sohw = ew.tile([128, 16], f32, tag="sohw")
                    # sig first: lets DVE start v = f*c while ACT does tanh(m)
                    nc.scalar.activation(sif[:], g1[:], ACTF.Sigmoid)
                    nc.scalar.activation(sm[:], g2[:], ACTF.Tanh)
                    nc.scalar.activation(sohw[:], g3[:], ACTF.Sigmoid)

                    u = ew.tile([128, 8], f32, tag="u")
                    v = ew.tile([128, 8], f32, tag="v")
                    nc.vector.tensor_mul(out=v[:], in0=sif[:, 8:16], in1=cbuf[:, t * 8:t * 8 + 8])
                    nc.vector.tensor_mul(out=u[:], in0=sif[:, 0:8], in1=sm[:])
                    nc.vector.tensor_add(out=cbuf[:, (t + 1) * 8:(t + 2) * 8], in0=u[:], in1=v[:])

                    tcn = ew.tile([128, 8], f32, tag="tcn")
                    nc.scalar.activation(tcn[:], cbuf[:, (t + 1) * 8:(t + 2) * 8], ACTF.Tanh)

                    pi6 = pich_r[:, 10:12, t, :]
                    a_ = ew.tile([128, 8], f32, tag="a_")
                    t1 = ew.tile([128, 8], f32, tag="t1")
                    bv = ew.tile([128, 8], f32, tag="bv")
                    w1 = ew.tile([128, 8], f32, tag="w1")
                    nc.vector.tensor_mul(out=a_[:], in0=sohw[:, 0:8], in1=sohw[:, 8:16])
                    nc.vector.tensor_mul(out=t1[:], in0=sohw[:, 8:16], in1=pi6)
                    nc.vector.tensor_sub(out=bv[:], in0=pi6, in1=t1[:])
                    nc.vector.tensor_mul(out=w1[:], in0=a_[:], in1=tcn[:])
                    # h (bf16, feeds next step's matmul) on the critical path
                    nc.vector.tensor_add(out=ybuf[:, (t + 1) * 8:(t + 2) * 8], in0=w1[:], in1=bv[:])
                    # exact fp32 y (off critical path)
                    nc.vector.tensor_add(out=youtb[:, t * 8:(t + 1) * 8], in0=w1[:], in1=bv[:])

                nc.sync.dma_start(out=yout_d[ck], in_=youtb[:])
                nc.sync.dma_start(out=cout_d[ck], in_=cbuf[:, 8:(CH + 1) * 8])

                # project next chunk AFTER the recurrence in PE program order:
                # its xt DMA has long completed, so no PE stall on DMA.
                if ck + 1 < nck:
                    pich_next = project(xt_next)

                ybuf_prev, cbuf_prev = ybuf, cbuf
                if ck + 1 < nck:
                    xt_cur, pich_cur = xt_next, pich_next

    nc.compile()
    return nc


_CACHE = {}


def _get_nc(nck):
    key = (nck, MODE)
    if key not in _CACHE:
        _CACHE[key] = _build(nck)
    return _CACHE[key]


def kernel(x, lengths, Wi, bi, Ws, bs):
    global LAST_RESULTS
    x = np.asarray(x, dtype=np.float32)
    lengths = np.asarray(lengths, dtype=np.int32)
    Wi = np.asarray(Wi, dtype=np.float32)
    bi = np.asarray(bi, dtype=np.float32)
    Ws = np.asarray(Ws, dtype=np.float32)
    bs = np.asarray(bs, dtype=np.float32)

    hdt = np.float32 if MODE == "fp32" else BF16
    maxlen = int(lengths.max())
    nck = max(1, (maxlen + CH - 1) // CH)

    # native gate order [i,f,m,o,hw,(x6)] matches the kernel layout
    Wi_r, bi_r, Ws_r, bs_r = Wi, bi, Ws, bs
    bfull = bi_r + np.concatenate([bs_r, np.zeros(H, np.float32)])

    # lhsT layouts: [k(part), (kh*NT+g)*128 + m]
    wst_h = np.ascontiguousarray(
        Ws_r.T.reshape(2, 128, 10, 128).transpose(1, 0, 2, 3).reshape(128, 2560)
    ).astype(hdt)
    wit_h = np.ascontiguousarray(
        Wi_r.T.reshape(2, 128, 12, 128).transpose(1, 0, 2, 3).reshape(128, 3072)
    ).astype(hdt)
    bias_h = np.ascontiguousarray(bfull.reshape(12, 128).T).astype(np.float32)
    iden_h = np.eye(128, dtype=np.float32).astype(hdt)

    in_maps = []
    for m in range(N_CORES):
        xs = x[m * BL:(m + 1) * BL, : nck * CH]          # [4, nck*CH, 256]
        # -> [nck, 2(ih), 128(i), CH, 4(b)]
        xt = xs.transpose(2, 1, 0).reshape(2, 128, nck, CH, BL).transpose(2, 0, 1, 3, 4)
        xt = np.ascontiguousarray(xt.reshape(nck, 2, 128, CH * BL)).astype(hdt)
        in_maps.append(
            dict(xt=xt, wst=wst_h, wit=wit_h, bias=bias_h, iden=iden_h)
        )

    nc = _get_nc(nck)
    res = run_bass_kernel_spmd(nc, in_maps, list(range(N_CORES)))
    LAST_RESULTS = res

    y = np.zeros((B, T, H), dtype=np.float32)
    h_f = np.zeros((B, H), dtype=np.float32)
    c_f = np.zeros((B, H), dtype=np.float32)
    for m in range(N_CORES):
        yo = res.results[m]["yout"].reshape(nck, 128, CH, 2, BL)
        co = res.results[m]["cout"].reshape(nck, 128, CH, 2, BL)
        for bl in range(BL):
            b = m * BL + bl
            L = int(lengths[b])
            # [nck,128,CH,2] -> [nck,CH,2,128] -> [nck*CH, 256]
            arr = yo[..., bl].transpose(0, 2, 3, 1).reshape(nck * CH, 256)
            y[b, :L] = arr[:L]
            h_f[b] = arr[L - 1]
            c_f[b] = co[..., bl].transpose(0, 2, 3, 1).reshape(nck * CH, 256)[L - 1]
    return y, h_f[None], c_f[None]


# revision 11
# speedup vs baseline: 230.0491x; 230.0491x over previous
"""CustomLSTM (highway) Trainium2 kernel.

B=32, T=2048, I=H=256. Data-parallel over batch: 8 cores x 4 rows.
Per core: fused input projection + serial recurrence, gates on partitions.

Layouts (per core, per chunk of CH=128 timesteps):
  pichunk  SBUF [128, 12*CH*4]  bf16: col = g*(CH*4) + t*4 + b
           gate-tile order g: [i0,i1,f0,f1,m0,m1,o0,o1,hw0,hw1,x60,x61]
  ybuf     SBUF [128, (CH+1)*8] bf16: h history, col = t*8 + kh*4 + b (slot0=carry)
  cbuf     SBUF [128, (CH+1)*8] f32 : c history, same layout
  youtbuf  SBUF [128, CH*8]     f32 : exact y, col = t*8 + kh*4 + b
Recurrence matmul: out[gate_tile 128, (b)] += WsT_tile[k=128, m=128].T @ h[k=128, b=4]
pi pre-added into PSUM via identity matmuls (i/f and m groups; o/hw group's
pi is added on the vector engine off the critical path). PSUM split in three
banks (i,f | m | o,hw) so sigmoid(i,f) fires after 8 of 20 weight matmuls.
Host extracts y[b, :len[b]], h_f = y[b, len-1], c_f from c history.
"""
import os
import sys

for _p in ("/opt/trn_rl_repo", "/root/.axon_site/_ro/trn_rl_repo"):
    if os.path.isdir(_p) and _p not in sys.path:
        sys.path.insert(0, _p)

import numpy as np
import ml_dtypes

import concourse.bass as bass
import concourse.mybir as mybir
from concourse import bacc
from concourse.tile import TileContext
from concourse.bass_utils import run_bass_kernel_spmd

BF16 = ml_dtypes.bfloat16
N_CORES = 8
B, T, I, H = 32, 2048, 256, 256
BL = B // N_CORES          # rows per core = 4
CH = 128                   # timesteps per chunk
MODE = os.environ.get("LSTM_MODE", "bf16")  # "bf16" | "fp32"

LAST_RESULTS = None        # test.py introspection


def _build(nck: int):
    fp32 = MODE == "fp32"
    wdt = mybir.dt.float32 if fp32 else mybir.dt.bfloat16
    nc = bacc.Bacc()
    ACTF = mybir.ActivationFunctionType

    xt_d = nc.declare_dram_parameter("xt", [nck, 2, 128, CH * BL], wdt, isOutput=False)
    wst_d = nc.declare_dram_parameter("wst", [128, 2560], wdt, isOutput=False)
    wit_d = nc.declare_dram_parameter("wit", [128, 3072], wdt, isOutput=False)
    bias_d = nc.declare_dram_parameter("bias", [128, 12], mybir.dt.float32, isOutput=False)
    iden_d = nc.declare_dram_parameter("iden", [128, 128], wdt, isOutput=False)
    yout_d = nc.declare_dram_parameter("yout", [nck, 128, CH * 8], mybir.dt.float32, isOutput=True)
    cout_d = nc.declare_dram_parameter("cout", [nck, 128, CH * 8], mybir.dt.float32, isOutput=True)

    f32 = mybir.dt.float32
    GW = CH * BL  # 512: per-gate-tile chunk width

    with TileContext(nc) as tc:
        with (
            tc.tile_pool(name="consts", bufs=1) as consts,
            tc.tile_pool(name="pich", bufs=2) as pich_pool,
            tc.tile_pool(name="xt", bufs=2) as xt_pool,
            tc.tile_pool(name="state", bufs=2) as state_pool,
            tc.tile_pool(name="ew", bufs=4) as ew,
            tc.tile_pool(name="g1", bufs=2, space="PSUM") as g1_pool,
            tc.tile_pool(name="g2", bufs=2, space="PSUM") as g2_pool,
            tc.tile_pool(name="g3", bufs=2, space="PSUM") as g3_pool,
            tc.tile_pool(name="pp", bufs=2, space="PSUM") as pp_pool,
        ):
            wst = consts.tile([128, 2560], wdt)
            wit = consts.tile([128, 3072], wdt)
            bias = consts.tile([128, 12], f32)
            iden = consts.tile([128, 128], wdt)
            nc.sync.dma_start(out=wst[:], in_=wst_d[:])
            nc.sync.dma_start(out=wit[:], in_=wit_d[:])
            nc.sync.dma_start(out=bias[:], in_=bias_d[:])
            nc.sync.dma_start(out=iden[:], in_=iden_d[:])

            def load_x(ck):
                xt = xt_pool.tile([128, 2 * GW], wdt, tag="xt")
                nc.sync.dma_start(out=xt[:, 0:GW], in_=xt_d[ck, 0])
                nc.sync.dma_start(out=xt[:, GW:2 * GW], in_=xt_d[ck, 1])
                return xt

            def project(xt):
                """input projection for one chunk -> pichunk tile"""
                pich = pich_pool.tile([128, 12 * GW], wdt, tag="pich")
                for g in range(12):
                    pg = pp_pool.tile([128, GW], f32, tag="pp")
                    for ih in range(2):
                        nc.tensor.matmul(
                            pg[:],
                            wit[:, (ih * 12 + g) * 128:(ih * 12 + g + 1) * 128],
                            xt[:, ih * GW:(ih + 1) * GW],
                            start=(ih == 0), stop=(ih == 1),
                        )
                    nc.vector.tensor_scalar_add(
                        out=pich[:, g * GW:(g + 1) * GW], in0=pg[:],
                        scalar1=bias[:, g:g + 1],
                    )
                return pich

            xt_cur = load_x(0)
            pich_cur = project(xt_cur)

            ybuf_prev = None
            cbuf_prev = None

            for ck in range(nck):
                ybuf = state_pool.tile([128, (CH + 1) * 8], wdt, tag="ybuf")
                cbuf = state_pool.tile([128, (CH + 1) * 8], f32, tag="cbuf")
                youtb = state_pool.tile([128, CH * 8], f32, tag="youtb")
                if ck == 0:
                    nc.vector.memset(ybuf[:, 0:8], 0.0)
                    nc.vector.memset(cbuf[:, 0:8], 0.0)
                else:
                    nc.vector.tensor_copy(out=ybuf[:, 0:8], in_=ybuf_prev[:, CH * 8:(CH + 1) * 8])
                    nc.vector.tensor_copy(out=cbuf[:, 0:8], in_=cbuf_prev[:, CH * 8:(CH + 1) * 8])

                # issue next chunk's x DMA early (completes during recurrence)
                if ck + 1 < nck:
                    xt_next = load_x(ck + 1)

                pich_r = pich_cur.rearrange("p (g t b) -> p g t b", g=12, t=CH, b=BL)

                for t in range(CH):
                    g1 = g1_pool.tile([128, 16], f32, tag="g1")   # i,f
                    g2 = g2_pool.tile([128, 8], f32, tag="g2")    # m
                    g3 = g3_pool.tile([128, 16], f32, tag="g3")   # o,hw
                    # pi pre-adds via identity matmul: all first, so the PE
                    # prefetches them while stalled on h(t-1)
                    nc.tensor.matmul(g1[:], iden[:], pich_r[:, 0:4, t, :], start=True, stop=False)
                    nc.tensor.matmul(g2[:], iden[:], pich_r[:, 4:6, t, :], start=True, stop=False)
                    # Ws matmuls in gate order i0,i1,f0,f1 | m0,m1 | o0,o1,hw0,hw1
                    for g in range(10):
                        if g < 4:
                            dst = g1[:, g * 4:(g + 1) * 4]
                        elif g < 6:
                            dst = g2[:, (g - 4) * 4:(g - 3) * 4]
                        else:
                            dst = g3[:, (g - 6) * 4:(g - 5) * 4]
                        for kh in range(2):
                            nc.tensor.matmul(
                                dst,
                                wst[:, (kh * 10 + g) * 128:(kh * 10 + g + 1) * 128],
                                ybuf[:, t * 8 + kh * 4: t * 8 + kh * 4 + 4],
                                start=(g == 6 and kh == 0),
                                stop=(kh == 1 and g in (3, 5, 9)),
                            )

                    sm = ew.tile([128, 8], f32, tag="sm")
                    sif = ew.tile([128, 16], f32, tag="sif")
                    sohw = ew.tile([128, 16], f32, tag="sohw")
                    # sig first: lets DVE start v = f*c while ACT does tanh(m)
                    nc.scalar.activation(sif[:], g1[:], ACTF.Sigmoid)
                    nc.scalar.activation(sm[:], g2[:], ACTF.Tanh)

                    u = ew.tile([128, 8], f32, tag="u")
                    v = ew.tile([128, 8], f32, tag="v")
                    nc.vector.tensor_mul(out=v[:], in0=sif[:, 8:16], in1=cbuf[:, t * 8:t * 8 + 8])
                    nc.vector.tensor_mul(out=u[:], in0=sif[:, 0:8], in1=sm[:])
                    nc.vector.tensor_add(out=cbuf[:, (t + 1) * 8:(t + 2) * 8], in0=u[:], in1=v[:])
                    # o/hw pi-add on DVE (off critical path; saves a PE pair)
                    g3s = ew.tile([128, 16], f32, tag="g3s")
                    nc.vector.tensor_add(out=g3s[:], in0=g3[:], in1=pich_r[:, 6:10, t, :])
                    nc.scalar.activation(sohw[:], g3s[:], ACTF.Sigmoid)

                    tcn = ew.tile([128, 8], f32, tag="tcn")
                    nc.scalar.activation(tcn[:], cbuf[:, (t + 1) * 8:(t + 2) * 8], ACTF.Tanh)

                    pi6 = pich_r[:, 10:12, t, :]
                    a_ = ew.tile([128, 8], f32, tag="a_")
                    t1 = ew.tile([128, 8], f32, tag="t1")
                    bv = ew.tile([128, 8], f32, tag="bv")
                    w1 = ew.tile([128, 8], f32, tag="w1")
                    nc.vector.tensor_mul(out=a_[:], in0=sohw[:, 0:8], in1=sohw[:, 8:16])
                    nc.vector.tensor_mul(out=t1[:], in0=sohw[:, 8:16], in1=pi6)
                    nc.vector.tensor_sub(out=bv[:], in0=pi6, in1=t1[:])
                    nc.vector.tensor_mul(out=w1[:], in0=a_[:], in1=tcn[:])
                    # h (bf16, feeds next step's matmul) on the critical path
                    nc.vector.tensor_add(out=ybuf[:, (t + 1) * 8:(t + 2) * 8], in0=w1[:], in1=bv[:])
                    # exact fp32 y (off critical path)
                    nc.vector.tensor_add(out=youtb[:, t * 8:(t + 1) * 8], in0=w1[:], in1=bv[:])

                nc.sync.dma_start(out=yout_d[ck], in_=youtb[:])
                nc.sync.dma_start(out=cout_d[ck], in_=cbuf[:, 8:(CH + 1) * 8])

                # project next chunk AFTER the recurrence in PE program order:
                # its xt DMA has long completed, so no PE stall on DMA.
                if ck + 1 < nck:
                    pich_next = project(xt_next)

                ybuf_prev, cbuf_prev = ybuf, cbuf
                if ck + 1 < nck:
                    xt_cur, pich_cur = xt_next, pich_next

    nc.compile()
    return nc


_CACHE = {}


def _get_nc(nck):
    key = (nck, MODE)
    if key not in _CACHE:
        _CACHE[key] = _build(nck)
    return _CACHE[key]


def kernel(x, lengths, Wi, bi, Ws, bs):
    global LAST_RESULTS
    x = np.asarray(x, dtype=np.float32)
    lengths = np.asarray(lengths, dtype=np.int32)
    Wi = np.asarray(Wi, dtype=np.float32)
    bi = np.asarray(bi, dtype=np.float32)
    Ws = np.asarray(Ws, dtype=np.float32)
    bs = np.asarray(bs, dtype=np.float32)

    hdt = np.float32 if MODE == "fp32" else BF16
    maxlen = int(lengths.max())
    nck = max(1, (maxlen + CH - 1) // CH)

    # native gate order [i,f,m,o,hw,(x6)] matches the kernel layout
    Wi_r, bi_r, Ws_r, bs_r = Wi, bi, Ws, bs
    bfull = bi_r + np.concatenate([bs_r, np.zeros(H, np.float32)])

    # lhsT layouts: [k(part), (kh*NT+g)*128 + m]
    wst_h = np.ascontiguousarray(
        Ws_r.T.reshape(2, 128, 10, 128).transpose(1, 0, 2, 3).reshape(128, 2560)
    ).astype(hdt)
    wit_h = np.ascontiguousarray(
        Wi_r.T.reshape(2, 128, 12, 128).transpose(1, 0, 2, 3).reshape(128, 3072)
    ).astype(hdt)
    bias_h = np.ascontiguousarray(bfull.reshape(12, 128).T).astype(np.float32)
    iden_h = np.eye(128, dtype=np.float32).astype(hdt)

    in_maps = []
    for m in range(N_CORES):
        xs = x[m * BL:(m + 1) * BL, : nck * CH]          # [4, nck*CH, 256]
        # -> [nck, 2(ih), 128(i), CH, 4(b)]
        xt = xs.transpose(2, 1, 0).reshape(2, 128, nck, CH, BL).transpose(2, 0, 1, 3, 4)
        xt = np.ascontiguousarray(xt.reshape(nck, 2, 128, CH * BL)).astype(hdt)
        in_maps.append(
            dict(xt=xt, wst=wst_h, wit=wit_h, bias=bias_h, iden=iden_h)
        )

    nc = _get_nc(nck)
    res = run_bass_kernel_spmd(nc, in_maps, list(range(N_CORES)))
    LAST_RESULTS = res

    y = np.zeros((B, T, H), dtype=np.float32)
    h_f = np.zeros((B, H), dtype=np.float32)
    c_f = np.zeros((B, H), dtype=np.float32)
    for m in range(N_CORES):
        yo = res.results[m]["yout"].reshape(nck, 128, CH, 2, BL)
        co = res.results[m]["cout"].reshape(nck, 128, CH, 2, BL)
        for bl in range(BL):
            b = m * BL + bl
            L = int(lengths[b])
            # [nck,128,CH,2] -> [nck,CH,2,128] -> [nck*CH, 256]
            arr = yo[..., bl].transpose(0, 2, 3, 1).reshape(nck * CH, 256)
            y[b, :L] = arr[:L]
            h_f[b] = arr[L - 1]
            c_f[b] = co[..., bl].transpose(0, 2, 3, 1).reshape(nck * CH, 256)[L - 1]
    return y, h_f[None], c_f[None]
